# revision 11
# baseline (speedup 1.0000x reference)
"""GAT (2-layer, heads=1) on 8 Trainium2 NeuronCores.

Strategy (1D node partition): nodes split into 8 chunks of NL; core c owns
dst-chunk c. Per layer, node tables (h|alpha_src packed 2xbf16 per fp32 word)
are computed feature-major via PE matmuls, AllGathered, and held in SBUF:
partition group g (16 partitions) holds chunk g's table. Edges are bucketed
on the host by (dst-core, src-chunk, dst-block) and sorted by dst; each edge
tile covers one NB-wide dst block for all 8 src-chunk groups. Per-edge
gathers run on GPSIMD (ap_gather), the logit add + exp-weighted h on DVE/ACT,
and segment sums via UNMASKED fp32 prefix scans + boundary-difference:
cum[last(d)] - cum[first(d)-1], folded into the PE group-combine as
(+sel)x(end) + (-sel)x(begin) PSUM accumulation; self-loop terms are
PE-accumulated from precomputed per-node tables. Per-node softmax
normalization (reciprocal) is batched per layer as ACT ln/exp passes.
Host preprocessing only reorders/buckets edge indices and relayouts x.
"""

import math
import numpy as np

import ml_dtypes

from concourse import bass, bacc, mybir
import concourse.tile as tile

F32 = mybir.dt.float32
BF16 = mybir.dt.bfloat16
I16 = mybir.dt.int16

NEG_SLOPE = 0.2
# fp32 -29952.0 == 0xC6EA0000: low bf16 lane (h) = 0, high bf16 lane
# (alpha_src) = -29952 -> exp(lrelu(...)) == 0, so sentinel edges vanish.
SENTINEL_PACKED = -29952.0

FULL_CFG = dict(
    NCORES=8, N=100000, F=512, H=16,
    NL=12500, NB=500, NT=25, CH=500, NCH=25,
)

SMALL_CFG = dict(
    NCORES=8, N=12800, F=512, H=16,
    NL=1600, NB=400, NT=4, CH=400, NCH=4,
)


# ---------------------------------------------------------------- host prep

def _round_up(x, m):
    return (x + m - 1) // m * m


def host_prep(edge_index, cfg):
    """Bucket and sort edges; build device index streams.

    Streams per (core, group, block) bucket, all front-padded by 2 slots:
      srcl: chunk-local src (sentinel NL), dstl: block-local dst (sent. NB),
      bnd:  per-dst position of its last edge (default T_e-1),
      bndp: per-dst position before its first edge (default T_e-1).
    Returns (T_e, per_core) with per_core[c]['idxs'] = [128, NT*IW] i16.
    """
    NC, N, NL = cfg["NCORES"], cfg["N"], cfg["NL"]
    NB, NT = cfg["NB"], cfg["NT"]
    G = 8
    PAD = 2

    src = np.asarray(edge_index[0], dtype=np.int64)
    dst = np.asarray(edge_index[1], dtype=np.int64)

    core = dst // NL
    grp = src // NL
    blk = (dst % NL) // NB
    order = np.lexsort((src, dst, grp, core))
    src, dst, core, grp, blk = (a[order] for a in (src, dst, core, grp, blk))

    bucket = (core * G + grp) * NT + blk
    nbuck = NC * G * NT
    counts = np.bincount(bucket, minlength=nbuck)
    T_e = _round_up(int(counts.max()) + PAD + 16, 128)
    assert T_e <= 32767

    starts = np.zeros(nbuck, dtype=np.int64)
    starts[1:] = np.cumsum(counts)[:-1]
    pos = np.arange(src.size) - starts[bucket] + PAD

    same = (bucket[1:] == bucket[:-1]) & (dst[1:] == dst[:-1])
    is_last = np.ones(src.size, dtype=bool)
    is_last[:-1] = ~same
    is_first = np.ones(src.size, dtype=bool)
    is_first[1:] = ~same

    srcl = np.full((NC, G, NT, T_e), NL, dtype=np.int16)
    dstl = np.zeros((NC, G, NT, T_e), dtype=np.int16)
    bnd = np.full((NC, G, NT, 512), T_e - 1, dtype=np.int16)
    bndp = np.full((NC, G, NT, 512), T_e - 1, dtype=np.int16)

    # dst indices address the alpha_dst region of the combined gather table:
    # region r = t%4 lives at NLP + r*ADW (NLP = NL+16, ADW = NB+4).
    NLP, ADW = NL + 16, NB + 4
    roff = NLP + (np.arange(NT, dtype=np.int64) % 4) * ADW  # per-tile offset
    c_, g_, b_ = core, grp, blk
    srcl[c_, g_, b_, pos] = (src % NL).astype(np.int16)
    dstl[:] = NLP + (np.arange(NT, dtype=np.int16)[None, None, :, None] % 4) * ADW + NB
    dstl[c_, g_, b_, pos] = (roff[b_] + (dst % NL) % NB).astype(np.int16)
    dl = ((dst % NL) % NB)
    bnd[c_[is_last], g_[is_last], b_[is_last],
        dl[is_last]] = pos[is_last].astype(np.int16)
    bndp[c_[is_first], g_[is_first], b_[is_first],
         dl[is_first]] = (pos[is_first] - 1).astype(np.int16)

    def wrap(a):
        # [NC, G, NT, w] -> [NC, 128, NT, w//16]; out[c, 16g+p, t, s] = a[c, g, t, s*16+p]
        n = a.shape[-1]
        return (a.reshape(NC, G, NT, n // 16, 16)
                 .transpose(0, 1, 4, 2, 3)
                 .reshape(NC, 128, NT, n // 16))

    idxs = np.concatenate(
        [wrap(srcl), wrap(dstl), wrap(bnd), wrap(bndp)], axis=3)
    IW = idxs.shape[3]
    idxs = np.ascontiguousarray(idxs.reshape(NC, 128, NT * IW))
    per_core = [{"idxs": idxs[c]} for c in range(NC)]
    return T_e, per_core


# ------------------------------------------------------------- device build

def build_nc(cfg, T_e, max_waits=2, ctrl_max_waits=1, split=True):
    NC, N, F, H = cfg["NCORES"], cfg["N"], cfg["F"], cfg["H"]
    NL, NB, NT = cfg["NL"], cfg["NB"], cfg["NT"]
    CH, NCH = cfg["CH"], cfg["NCH"]
    KB = F // 128
    NLP = NL + 16               # table width incl. sentinel
    ADW = NB + 4                # per-tile alpha_dst table width
    T16 = T_e // 16
    IW = 2 * T16 + 64
    NBLK = math.ceil(NL / 128)
    NL2 = NBLK * 128
    NRMB = 5                    # chunks in the normalization pass
    assert NL % NRMB == 0
    rg = [list(range(NC))]

    nc = bacc.Bacc("TRN2", target_bir_lowering=False)

    xth = nc.declare_dram_parameter("xth", [128, NCH * KB * CH], F32, isOutput=False)
    w1 = nc.declare_dram_parameter("w1", [F, H], F32, isOutput=False)
    w2 = nc.declare_dram_parameter("w2", [H, H], F32, isOutput=False)
    a1rep = nc.declare_dram_parameter("a1rep", [H, 16], F32, isOutput=False)
    ad1rep = nc.declare_dram_parameter("ad1rep", [H, 16], F32, isOutput=False)
    a2rep = nc.declare_dram_parameter("a2rep", [H, 16], F32, isOutput=False)
    ad2rep = nc.declare_dram_parameter("ad2rep", [H, 16], F32, isOutput=False)
    b1p = nc.declare_dram_parameter("b1p", [H, 1], F32, isOutput=False)
    b2p = nc.declare_dram_parameter("b2p", [H, 1], F32, isOutput=False)
    selp = nc.declare_dram_parameter("selp", [128, 16], F32, isOutput=False)
    selnp = nc.declare_dram_parameter("selnp", [128, 16], F32, isOutput=False)
    id16p = nc.declare_dram_parameter("id16p", [16, 16], BF16, isOutput=False)
    identp = nc.declare_dram_parameter("identp", [16, 16], F32, isOutput=False)
    idxsp = nc.declare_dram_parameter("idxs", [128, NT * IW], I16, isOutput=False)
    outp = nc.declare_dram_parameter("out", [128, NBLK * H], F32, isOutput=True)

    ag_in = [nc.dram_tensor(f"ag_in{l}", [16, NL], F32) for l in (1, 2)]
    ag_out = [nc.dram_tensor(f"ag_out{l}", [128, NL], F32, addr_space="Shared")
              for l in (1, 2)]
    ad_nl = [nc.dram_tensor(f"ad_nl{l}", [1, NL + 32], F32) for l in (1, 2)]
    pw_nl = [nc.dram_tensor(f"pw_nl{l}", [16, NL], F32) for l in (1, 2)]
    numd = [nc.dram_tensor(f"numd{l}", [16, NL], F32) for l in (1, 2)]
    dend = [nc.dram_tensor(f"dend{l}", [16, NL], F32) for l in (1, 2)]
    uvd = [nc.dram_tensor(f"uvd{l}", [16, NL], F32) for l in (1, 2)]

    AF = mybir.ActivationFunctionType
    ALU = mybir.AluOpType

    with tile.TileContext(nc, num_cores=NC) as tc:
        with tc.tile_pool(name="const", bufs=1) as cpool:
            w1t = cpool.tile([128, KB, H], F32)
            nc.sync.dma_start(out=w1t[:], in_=w1[:].rearrange("(b p) h -> p b h", p=128))
            w2t = cpool.tile([16, H], F32)
            nc.sync.dma_start(out=w2t[:], in_=w2[:])
            a1t = cpool.tile([16, 16], F32)
            nc.sync.dma_start(out=a1t[:], in_=a1rep[:])
            ad1t = cpool.tile([16, 16], F32)
            nc.sync.dma_start(out=ad1t[:], in_=ad1rep[:])
            a2t = cpool.tile([16, 16], F32)
            nc.sync.dma_start(out=a2t[:], in_=a2rep[:])
            ad2t = cpool.tile([16, 16], F32)
            nc.sync.dma_start(out=ad2t[:], in_=ad2rep[:])
            b1t = cpool.tile([16, 1], F32)
            nc.sync.dma_start(out=b1t[:], in_=b1p[:])
            b2t = cpool.tile([16, 1], F32)
            nc.sync.dma_start(out=b2t[:], in_=b2p[:])
            selt = cpool.tile([128, 16], F32)
            nc.sync.dma_start(out=selt[:], in_=selp[:])
            selnt = cpool.tile([128, 16], F32)
            nc.sync.dma_start(out=selnt[:], in_=selnp[:])
            id16t = cpool.tile([16, 16], BF16)
            nc.sync.dma_start(out=id16t[:], in_=id16p[:])
            idt = cpool.tile([16, 16], F32)
            nc.sync.dma_start(out=idt[:], in_=identp[:])
            ones1 = cpool.tile([1, 128], F32)
            nc.vector.memset(ones1[:], 1.0)
            zerob = cpool.tile([128, 1], BF16)
            nc.vector.memset(zerob[:], 0.0)
            zrow = cpool.tile([1, 32], F32)
            nc.vector.memset(zrow[:], 0.0)
            idxt = cpool.tile([128, NT * IW], I16)
            nc.sync.dma_start(out=idxt[:], in_=idxsp[:])

            # ---------------- phase A (tables) -----------------------------
            def phase_a(l, wt, at, adt, rhs_of):
                """Compute per-chunk (h|alpha_src) packed tables, alpha_dst
                row, and packed (wself|pself) self-loop tables; stage to DRAM.
                rhs_of(c, pa, pap) -> ([16, CH] f32 rhs AP, KB') for the
                h matmul: KB' accumulation steps."""
                with (
                    tc.tile_pool(name=f"pa{l}", bufs=2) as pa,
                    tc.tile_pool(name=f"pap{l}", bufs=2, space="PSUM") as pap,
                ):
                    for c in range(NCH):
                        sl = slice(c * CH, (c + 1) * CH)
                        ph = pap.tile([16, CH], F32, tag="ph")
                        rhs_list = rhs_of(c, pa)
                        for b, (lhsT, rhs) in enumerate(rhs_list):
                            nc.tensor.matmul(ph[:], lhsT=lhsT, rhs=rhs,
                                             start=(b == 0),
                                             stop=(b == len(rhs_list) - 1))
                        hch = pa.tile([16, CH], F32, tag="hch")
                        nc.scalar.activation(hch[:], ph[:], AF.Copy)
                        pas = pap.tile([16, CH], F32, tag="pas")
                        nc.tensor.matmul(pas[:], lhsT=at[:], rhs=hch[:],
                                         start=True, stop=True)
                        pad_ = pap.tile([16, CH], F32, tag="pad")
                        nc.tensor.matmul(pad_[:], lhsT=adt[:], rhs=hch[:],
                                         start=True, stop=True)
                        adfull = pa.tile([16, CH], F32, tag="adfull")
                        nc.vector.tensor_copy(adfull[:], pad_[:])
                        nc.sync.dma_start(out=ad_nl[l][0:1, sl], in_=adfull[0:1, :])
                        packed = pa.tile([16, CH], F32, tag="packed")
                        pb = packed[:].bitcast(BF16)
                        nc.scalar.activation(pb[:, 0::2], hch[:], AF.Copy)
                        nc.scalar.activation(pb[:, 1::2], pas[:], AF.Copy)
                        nc.sync.dma_start(out=ag_in[l][:, sl], in_=packed[:])
                        # analytic self-loop terms
                        tself = pa.tile([16, CH], F32, tag="tself")
                        nc.vector.tensor_add(tself[:], pas[:], adfull[:])
                        nc.vector.scalar_tensor_tensor(
                            tself[:], tself[:], NEG_SLOPE, tself[:],
                            op0=ALU.mult, op1=ALU.max)
                        pselfc = pa.tile([16, CH], F32, tag="pselfc")
                        nc.scalar.activation(pselfc[:], tself[:], AF.Exp)
                        wselfc = pa.tile([16, CH], F32, tag="wselfc")
                        nc.vector.tensor_mul(wselfc[:], pselfc[:], hch[:])
                        pwc = pa.tile([16, CH], F32, tag="pwc")
                        pwb = pwc[:].bitcast(BF16)
                        nc.vector.tensor_copy(pwb[:, 0::2], wselfc[:])
                        nc.vector.tensor_copy(pwb[:, 1::2], pselfc[:])
                        nc.sync.dma_start(out=pw_nl[l][:, sl], in_=pwc[:])
                    nc.sync.dma_start(out=ad_nl[l][0:1, NL:], in_=zrow[:])

            def rhs_a1(c, pa):
                xt_t = pa.tile([128, KB, CH], F32, tag="xt")
                nc.sync.dma_start(
                    out=xt_t[:],
                    in_=xth[:, c * KB * CH:(c + 1) * KB * CH]
                    .rearrange("p (b n) -> p b n", b=KB))
                return [(w1t[:, b, :], xt_t[:, b, :]) for b in range(KB)]

            # ---------------- edge phase -----------------------------------
            def run_layer(l, uv_writer):
                nc.gpsimd.collective_compute(
                    "AllGather", ALU.bypass, replica_groups=rg,
                    ins=[ag_in[l][:]], outs=[ag_out[l][:]])
                TBW = NLP + 4 * ADW
                with tc.tile_pool(name=f"tab{l}", bufs=1) as tabp:
                    table = tabp.tile([128, TBW], F32, tag="table")
                    nc.sync.dma_start(out=table[:, :NL], in_=ag_out[l][:])
                    nc.vector.memset(table[:, NL:], SENTINEL_PACKED)
                    with (
                        tc.tile_pool(name=f"ed{l}", bufs=2) as ed,
                        tc.tile_pool(name=f"eb{l}", bufs=3) as eb,
                        tc.tile_pool(name=f"edp{l}", bufs=2, space="PSUM") as edp,
                        tc.tile_pool(name=f"adp{l}", bufs=3, space="PSUM") as adp,
                    ):
                        def mk_adbs(t):
                            # alpha_dst block row -> PE rank-1 broadcast ->
                            # ACT copy into table region t%4.
                            ads = eb.tile([1, ADW], F32, tag="ads")
                            nc.sync.dma_start(
                                out=ads[:],
                                in_=ad_nl[l][0:1, t * NB: t * NB + ADW])
                            pws = eb.tile([16, NB], F32, tag="pws")
                            nc.sync.dma_start(
                                out=pws[:],
                                in_=pw_nl[l][:, t * NB:(t + 1) * NB])
                            adp_t = adp.tile([128, ADW], F32, tag="adb")
                            nc.tensor.matmul(
                                adp_t[:], lhsT=ones1[:], rhs=ads[:],
                                start=True, stop=True)
                            r = t % 4
                            nc.scalar.activation(
                                table[:, NLP + r * ADW: NLP + (r + 1) * ADW],
                                adp_t[:], AF.Copy)
                            return pws[:]

                        pws_q = [mk_adbs(0), mk_adbs(1) if NT > 1 else None]
                        for t in range(NT):
                            it = idxt[:, t * IW:(t + 1) * IW]
                            pwsl = pws_q[0]
                            ghs = ed.tile([128, 2 * T_e], F32, tag="ghs")
                            nc.gpsimd.ap_gather(
                                ghs[:], table[:], it[:, 0:2 * T16],
                                channels=128, num_elems=TBW, d=1,
                                num_idxs=2 * T_e)
                            gb = ghs[:, :T_e].bitcast(BF16)  # [128, 2*T_e]
                            adg = ghs[:, T_e:]
                            tt = ed.tile([128, T_e], BF16, tag="tt")
                            nc.vector.tensor_add(tt[:], gb[:, 1::2], adg)
                            nc.scalar.activation(tt[:], tt[:], AF.Lrelu,
                                                 alpha=NEG_SLOPE)
                            pp = ed.tile([128, T_e], BF16, tag="pp")
                            nc.scalar.activation(pp[:], tt[:], AF.Exp)
                            # prefetch alpha_dst region two tiles ahead
                            if t + 2 < NT:
                                pws_q.append(mk_adbs(t + 2))
                            nc.vector.tensor_mul(tt[:], pp[:], gb[:, 0::2])
                            sc = ed.tile([128, T_e, 2], F32, tag="sc")
                            nc.vector.tensor_tensor_scan(
                                sc[:, :, 0], zerob[:, 0:1].to_broadcast([128, T_e]),
                                tt[:], 0.0, ALU.add, ALU.add)
                            nc.vector.tensor_tensor_scan(
                                sc[:, :, 1], zerob[:, 0:1].to_broadcast([128, T_e]),
                                pp[:], 0.0, ALU.add, ALU.add)
                            bg = ed.tile([128, 1024, 2], F32, tag="bg")
                            nc.gpsimd.ap_gather(
                                bg[:], sc[:], it[:, 2 * T16:2 * T16 + 64],
                                channels=128, num_elems=T_e, d=2, num_idxs=1024)
                            pwb = pwsl.bitcast(BF16)  # [16, 2*NB]
                            pu = edp.tile([16, 512], F32, tag="pu")
                            nc.tensor.matmul(pu[:], lhsT=selt[:],
                                             rhs=bg[:, 0:512, 0],
                                             start=True, stop=False)
                            nc.tensor.matmul(pu[:], lhsT=selnt[:],
                                             rhs=bg[:, 512:1024, 0],
                                             start=False, stop=False)
                            nc.tensor.matmul(pu[:, :NB], lhsT=id16t[:],
                                             rhs=pwb[:, 0::2],
                                             start=False, stop=True)
                            ps = edp.tile([16, 512], F32, tag="ps")
                            nc.tensor.matmul(ps[:], lhsT=selt[:],
                                             rhs=bg[:, 0:512, 1],
                                             start=True, stop=False)
                            nc.tensor.matmul(ps[:], lhsT=selnt[:],
                                             rhs=bg[:, 512:1024, 1],
                                             start=False, stop=False)
                            nc.tensor.matmul(ps[:, :NB], lhsT=id16t[:],
                                             rhs=pwb[:, 1::2],
                                             start=False, stop=True)
                            puc = ed.tile([16, NB], F32, tag="puc")
                            nc.scalar.activation(puc[:], pu[:, :NB], AF.Copy)
                            psc = ed.tile([16, NB], F32, tag="psc")
                            nc.scalar.activation(psc[:], ps[:, :NB], AF.Copy)
                            nc.sync.dma_start(out=numd[l][:, t * NB:(t + 1) * NB],
                                              in_=puc[:])
                            nc.sync.dma_start(out=dend[l][:, t * NB:(t + 1) * NB],
                                              in_=psc[:])
                            pws_q.pop(0)

                # batched softmax normalization: uv = num * exp(-ln(den))
                NRM = NL // NRMB
                with tc.tile_pool(name=f"nm{l}", bufs=2) as nmp:
                    for c5 in range(NRMB):
                        sl = slice(c5 * NRM, (c5 + 1) * NRM)
                        nmt = nmp.tile([16, NRM], F32, tag="nmt")
                        nc.sync.dma_start(out=nmt[:], in_=numd[l][:, sl])
                        dnt = nmp.tile([16, NRM], F32, tag="dnt")
                        nc.sync.dma_start(out=dnt[:], in_=dend[l][:, sl])
                        ld = nmp.tile([16, NRM], F32, tag="ld")
                        nc.scalar.activation(ld[:], dnt[:], AF.Ln)
                        rc = nmp.tile([16, NRM], F32, tag="rc")
                        nc.scalar.activation(rc[:], ld[:], AF.Exp, scale=-1.0)
                        uvm = nmp.tile([16, NRM], F32, tag="uvm")
                        nc.vector.tensor_mul(uvm[:], nmt[:], rc[:])
                        uvw = nmp.tile([16, NRM], F32, tag="uvw")
                        uv_writer(uvw, uvm)
                        nc.sync.dma_start(out=uvd[l][:, sl], in_=uvw[:])

            # ======================= layer 1 ===============================
            phase_a(0, w1t, a1t, ad1t, rhs_a1)

            def write1(uvw, uvm):
                nc.vector.tensor_scalar(
                    uvw[:], uvm[:], b1t[:, 0:1], 0.0,
                    op0=ALU.add, op1=ALU.max)

            run_layer(0, write1)

            # phase A (layer 2) reads uv1 chunks back from DRAM
            def rhs_a2(c, pa):
                uc = pa.tile([16, CH], F32, tag="uc")
                nc.sync.dma_start(out=uc[:], in_=uvd[0][:, c * CH:(c + 1) * CH])
                return [(w2t[:], uc[:])]

            phase_a(1, w2t, a2t, ad2t, rhs_a2)

            # ======================= layer 2 ===============================
            def write2(uvw, uvm):
                nc.vector.tensor_scalar_add(uvw[:], uvm[:], b2t[:, 0:1])

            run_layer(1, write2)

            # ---------------- log_softmax + transpose + store -------------
            with (
                tc.tile_pool(name="fin", bufs=2) as fin,
                tc.tile_pool(name="finp", bufs=4, space="PSUM") as finp,
                tc.tile_pool(name="fino", bufs=1) as fino,
            ):
                uv2 = fino.tile([16, NL2], F32, tag="uv2")
                if NL2 > NL:
                    nc.vector.memset(uv2[:, NL:], 0.0)
                nc.sync.dma_start(out=uv2[:, :NL], in_=uvd[1][:])
                if True:
                    nodemaj = fino.tile([128, NBLK, H], F32, tag="nodemaj")
                    for j in range(NBLK):
                        ptp = finp.tile([128, 16], F32, tag="ptp")
                        nc.tensor.transpose(ptp[:], uv2[:, j * 128:(j + 1) * 128],
                                            idt[:])
                        nc.vector.tensor_copy(nodemaj[:, j, :], ptp[:, :H])
                    mx = fin.tile([128, NBLK], F32, tag="mx")
                    nc.vector.tensor_reduce(mx[:], nodemaj[:],
                                            axis=mybir.AxisListType.X,
                                            op=ALU.max)
                    zz = fino.tile([128, NBLK, H], F32, tag="zz")
                    nc.vector.tensor_sub(zz[:], nodemaj[:],
                                         mx[:, :, None].to_broadcast([128, NBLK, H]))
                    es = fino.tile([128, NBLK, H], F32, tag="es")
                    nc.scalar.activation(es[:], zz[:], AF.Exp)
                    sm = fin.tile([128, NBLK], F32, tag="sm")
                    nc.vector.tensor_reduce(sm[:], es[:],
                                            axis=mybir.AxisListType.X,
                                            op=ALU.add)
                    ls = fin.tile([128, NBLK], F32, tag="ls")
                    nc.scalar.activation(ls[:], sm[:], AF.Ln)
                    outf = fino.tile([128, NBLK, H], F32, tag="outf")
                    nc.vector.tensor_sub(outf[:], zz[:],
                                         ls[:, :, None].to_broadcast([128, NBLK, H]))
                    nc.sync.dma_start(out=outp[:].rearrange("p (b h) -> p b h", h=H),
                                      in_=outf[:])

    nc.compile()
    if split:
        split_waits(nc, max_waits=max_waits, ctrl_max_waits=ctrl_max_waits)
    return nc


CTRL_TYPES = ("InstDrain", "InstNoOp", "InstHalt", "InstEventSemaphore")


def split_waits(nc, max_waits=2, ctrl_max_waits=1):
    """walrus in this container caps sync-waits per instruction; move excess
    waits onto preceding same-engine NoOps (each carrying one wait)."""
    for f in nc.m.functions:
        for bb in f.blocks:
            new_insts, changed = [], False
            for ins in bb.instructions:
                si = ins.sync_info
                cap = (ctrl_max_waits if type(ins).__name__ in CTRL_TYPES
                       else max_waits)
                if si is not None and si.on_wait is not None and len(si.on_wait) > cap:
                    waits = list(si.on_wait)
                    excess, keep = waits[:-cap] if cap else waits, waits[-cap:] if cap else []
                    for i, w in enumerate(excess):
                        nop = mybir.InstNoOp(name=f"{ins.name}-ws{i}", ins=[], outs=[])
                        nop.engine = ins.engine
                        nop.sync_info = mybir.SyncInfo(on_wait=[w], on_update=[])
                        new_insts.append(nop)
                    si.on_wait = keep
                    changed = True
                new_insts.append(ins)
            if changed:
                bb.instructions = new_insts
    for f in nc.m.functions:
        for bb in f.blocks:
            for ins in bb.instructions:
                si = ins.sync_info
                cap = (ctrl_max_waits if type(ins).__name__ in CTRL_TYPES
                       else max_waits)
                assert si is None or si.on_wait is None or len(si.on_wait) <= cap, \
                    f"{ins.name}: {len(si.on_wait)} waits > {cap}"


# ------------------------------------------------------------ input packing

def make_in_maps(inputs, cfg, per_core):
    NC, NL, H, F = cfg["NCORES"], cfg["NL"], cfg["H"], cfg["F"]
    CH, NCH = cfg["CH"], cfg["NCH"]
    KB = F // 128
    x = np.asarray(inputs["x"], dtype=np.float32)
    # per-core layout [128, NCH, KB, CH]: xt_h[p, c, b, n] = x[n_glob, 128b+p]
    xt_full = np.ascontiguousarray(x.T)  # [F, N]
    sel = np.zeros((128, 16), dtype=np.float32)
    sel[np.arange(128), np.arange(128) % 16] = 1.0
    shared = {
        "w1": np.ascontiguousarray(np.asarray(inputs["W1"], np.float32)),
        "w2": np.ascontiguousarray(np.asarray(inputs["W2"], np.float32)),
        "a1rep": np.ascontiguousarray(np.repeat(np.asarray(inputs["a_src1"], np.float32)[:, None], 16, 1)),
        "ad1rep": np.ascontiguousarray(np.repeat(np.asarray(inputs["a_dst1"], np.float32)[:, None], 16, 1)),
        "a2rep": np.ascontiguousarray(np.repeat(np.asarray(inputs["a_src2"], np.float32)[:, None], 16, 1)),
        "ad2rep": np.ascontiguousarray(np.repeat(np.asarray(inputs["a_dst2"], np.float32)[:, None], 16, 1)),
        "b1p": np.ascontiguousarray(np.asarray(inputs["b1"], np.float32)[:, None]),
        "b2p": np.ascontiguousarray(np.asarray(inputs["b2"], np.float32)[:, None]),
        "selp": sel,
        "selnp": -sel,
        "id16p": np.eye(16, dtype=ml_dtypes.bfloat16),
        "identp": np.eye(16, dtype=np.float32),
    }
    in_maps = []
    for c in range(NC):
        m = dict(shared)
        xc = xt_full[:, c * NL:(c + 1) * NL]          # [F, NL]
        xc = xc.reshape(KB, 128, NCH, CH)             # [b, p, c, n]
        xc = xc.transpose(1, 2, 0, 3)                 # [p, c, b, n]
        m["xth"] = np.ascontiguousarray(xc.reshape(128, NCH * KB * CH))
        m["idxs"] = per_core[c]["idxs"]
        in_maps.append(m)
    return in_maps


def unshard_output(results, cfg):
    NC, NL, H = cfg["NCORES"], cfg["NL"], cfg["H"]
    NBLK = math.ceil(NL / 128)
    parts = []
    for c in range(NC):
        a = np.asarray(results[c]["out"]).reshape(128, NBLK, H)
        a = a.transpose(1, 0, 2).reshape(NBLK * 128, H)[:NL]
        parts.append(a)
    return np.concatenate(parts, axis=0)


# ------------------------------------------------------------------- driver

_CACHE = {}


def run_on_hw(inputs, cfg, trace=False, tmpdir=None):
    import os
    import shutil
    from concourse.bass_utils import run_bass_kernel_spmd
    if tmpdir is not None and os.path.isdir(tmpdir):
        shutil.rmtree(tmpdir, ignore_errors=True)
    if tmpdir is not None:
        os.makedirs(tmpdir, exist_ok=True)
    T_e, per_core = host_prep(inputs["edge_index"], cfg)
    key = (cfg["N"], T_e)
    if key not in _CACHE:
        _CACHE[key] = build_nc(cfg, T_e)
    nc = _CACHE[key]
    in_maps = make_in_maps(inputs, cfg, per_core)
    res = run_bass_kernel_spmd(nc, in_maps, list(range(cfg["NCORES"])),
                               trace=trace, tmpdir=tmpdir)
    out = unshard_output(res.results, cfg)
    return out, res


def kernel(**inputs):
    out, _ = run_on_hw(inputs, FULL_CFG)
    return out.astype(np.float32)


# revision 14
# speedup vs baseline: 1.1887x; 1.1887x over previous
"""GAT (2-layer, heads=1) on 8 Trainium2 NeuronCores.

Strategy (1D node partition): nodes split into 8 chunks of NL; core c owns
dst-chunk c. Per layer, node tables (h|alpha_src packed 2xbf16 per fp32 word)
are computed feature-major via PE matmuls, AllGathered, and held in SBUF:
partition group g (16 partitions) holds chunk g's table. Edges are bucketed
on the host by (dst-core, src-chunk, dst-block) and sorted by dst; each edge
tile covers one NB-wide dst block for all 8 src-chunk groups. Per-edge
gathers run on GPSIMD (ap_gather), the logit add + exp-weighted h on DVE/ACT,
and segment sums via UNMASKED fp32 prefix scans + boundary-difference:
cum[last(d)] - cum[first(d)-1], folded into the PE group-combine as
(+sel)x(end) + (-sel)x(begin) PSUM accumulation; self-loop terms are
PE-accumulated from precomputed per-node tables. Per-node softmax
normalization (reciprocal) is batched per layer as ACT ln/exp passes.
Host preprocessing only reorders/buckets edge indices and relayouts x.
"""

import math
import numpy as np

import ml_dtypes

from concourse import bass, bacc, mybir
import concourse.tile as tile

F32 = mybir.dt.float32
BF16 = mybir.dt.bfloat16
I16 = mybir.dt.int16

NEG_SLOPE = 0.2
# fp32 -29952.0 == 0xC6EA0000: low bf16 lane (h) = 0, high bf16 lane
# (alpha_src) = -29952 -> exp(lrelu(...)) == 0, so sentinel edges vanish.
SENTINEL_PACKED = -29952.0

FULL_CFG = dict(
    NCORES=8, N=100000, F=512, H=16,
    NL=12500, NB=500, NT=25, CH=500, NCH=25,
)

SMALL_CFG = dict(
    NCORES=8, N=12800, F=512, H=16,
    NL=1600, NB=400, NT=4, CH=400, NCH=4,
)


# ---------------------------------------------------------------- host prep

def _round_up(x, m):
    return (x + m - 1) // m * m


def host_prep(edge_index, cfg):
    """Bucket and sort edges; build device index streams.

    Streams per (core, group, block) bucket, all front-padded by 2 slots:
      srcl: chunk-local src (sentinel NL), dstl: block-local dst (sent. NB),
      bnd:  per-dst position of its last edge (default T_e-1),
      bndp: per-dst position before its first edge (default T_e-1).
    Returns (T_e, per_core) with per_core[c]['idxs'] = [128, NT*IW] i16.
    """
    NC, N, NL = cfg["NCORES"], cfg["N"], cfg["NL"]
    NB, NT = cfg["NB"], cfg["NT"]
    G = 8
    PAD = 2

    src = np.asarray(edge_index[0], dtype=np.int64)
    dst = np.asarray(edge_index[1], dtype=np.int64)

    core = dst // NL
    grp = src // NL
    blk = (dst % NL) // NB
    order = np.lexsort((src, dst, grp, core))
    src, dst, core, grp, blk = (a[order] for a in (src, dst, core, grp, blk))

    bucket = (core * G + grp) * NT + blk
    nbuck = NC * G * NT
    counts = np.bincount(bucket, minlength=nbuck)
    T_e = _round_up(int(counts.max()) + PAD + 16, 128)
    assert T_e <= 32767

    starts = np.zeros(nbuck, dtype=np.int64)
    starts[1:] = np.cumsum(counts)[:-1]
    pos = np.arange(src.size) - starts[bucket] + PAD

    same = (bucket[1:] == bucket[:-1]) & (dst[1:] == dst[:-1])
    is_last = np.ones(src.size, dtype=bool)
    is_last[:-1] = ~same
    is_first = np.ones(src.size, dtype=bool)
    is_first[1:] = ~same

    srcl = np.full((NC, G, NT, T_e), NL, dtype=np.int16)
    dstl = np.zeros((NC, G, NT, T_e), dtype=np.int16)
    bnd = np.full((NC, G, NT, 512), T_e - 1, dtype=np.int16)
    bndp = np.full((NC, G, NT, 512), T_e - 1, dtype=np.int16)

    c_, g_, b_ = core, grp, blk
    srcl[c_, g_, b_, pos] = (src % NL).astype(np.int16)
    dstl[:] = NB
    dstl[c_, g_, b_, pos] = ((dst % NL) % NB).astype(np.int16)
    dl = ((dst % NL) % NB)
    bnd[c_[is_last], g_[is_last], b_[is_last],
        dl[is_last]] = pos[is_last].astype(np.int16)
    bndp[c_[is_first], g_[is_first], b_[is_first],
         dl[is_first]] = (pos[is_first] - 1).astype(np.int16)

    def wrap(a):
        # [NC, G, NT, w] -> [NC, 128, NT, w//16]; out[c, 16g+p, t, s] = a[c, g, t, s*16+p]
        n = a.shape[-1]
        return (a.reshape(NC, G, NT, n // 16, 16)
                 .transpose(0, 1, 4, 2, 3)
                 .reshape(NC, 128, NT, n // 16))

    idxs = np.concatenate(
        [wrap(srcl), wrap(dstl), wrap(bnd), wrap(bndp)], axis=3)
    IW = idxs.shape[3]
    idxs = np.ascontiguousarray(idxs.reshape(NC, 128, NT * IW))
    per_core = [{"idxs": idxs[c]} for c in range(NC)]
    return T_e, per_core


# ------------------------------------------------------------- device build

def build_nc(cfg, T_e, max_waits=2, ctrl_max_waits=1, split=True):
    NC, N, F, H = cfg["NCORES"], cfg["N"], cfg["F"], cfg["H"]
    NL, NB, NT = cfg["NL"], cfg["NB"], cfg["NT"]
    CH, NCH = cfg["CH"], cfg["NCH"]
    KB = F // 128
    NLP = NL + 16               # table width incl. sentinel
    ADW = NB + 4                # per-tile alpha_dst table width
    T16 = T_e // 16
    IW = 2 * T16 + 64
    NBLK = math.ceil(NL / 128)
    NL2 = NBLK * 128
    NRMB = 5                    # chunks in the normalization pass
    assert NL % NRMB == 0
    rg = [list(range(NC))]

    nc = bacc.Bacc("TRN2", target_bir_lowering=False)

    xth = nc.declare_dram_parameter("xth", [128, NCH * KB * CH], F32, isOutput=False)
    w1 = nc.declare_dram_parameter("w1", [F, H], F32, isOutput=False)
    w2 = nc.declare_dram_parameter("w2", [H, H], F32, isOutput=False)
    a1rep = nc.declare_dram_parameter("a1rep", [H, 16], F32, isOutput=False)
    ad1rep = nc.declare_dram_parameter("ad1rep", [H, 16], F32, isOutput=False)
    a2rep = nc.declare_dram_parameter("a2rep", [H, 16], F32, isOutput=False)
    ad2rep = nc.declare_dram_parameter("ad2rep", [H, 16], F32, isOutput=False)
    b1p = nc.declare_dram_parameter("b1p", [H, 1], F32, isOutput=False)
    b2p = nc.declare_dram_parameter("b2p", [H, 1], F32, isOutput=False)
    selp = nc.declare_dram_parameter("selp", [128, 16], F32, isOutput=False)
    selnp = nc.declare_dram_parameter("selnp", [128, 16], F32, isOutput=False)
    id16p = nc.declare_dram_parameter("id16p", [16, 16], BF16, isOutput=False)
    identp = nc.declare_dram_parameter("identp", [16, 16], F32, isOutput=False)
    idxsp = nc.declare_dram_parameter("idxs", [128, NT * IW], I16, isOutput=False)
    outp = nc.declare_dram_parameter("out", [128, NBLK * H], F32, isOutput=True)

    ag_in = [nc.dram_tensor(f"ag_in{l}", [16, NL], F32) for l in (1, 2)]
    ag_out = [nc.dram_tensor(f"ag_out{l}", [128, NL], F32, addr_space="Shared")
              for l in (1, 2)]
    ad_nl = [nc.dram_tensor(f"ad_nl{l}", [1, NL + 32], F32) for l in (1, 2)]
    pw_nl = [nc.dram_tensor(f"pw_nl{l}", [16, NL], F32) for l in (1, 2)]
    numd = [nc.dram_tensor(f"numd{l}", [16, NL], F32) for l in (1, 2)]
    dend = [nc.dram_tensor(f"dend{l}", [16, NL], F32) for l in (1, 2)]
    uvd = [nc.dram_tensor(f"uvd{l}", [16, NL], F32) for l in (1, 2)]

    AF = mybir.ActivationFunctionType
    ALU = mybir.AluOpType

    with tile.TileContext(nc, num_cores=NC) as tc:
        with tc.tile_pool(name="const", bufs=1) as cpool:
            w1t = cpool.tile([128, KB, H], F32)
            nc.sync.dma_start(out=w1t[:], in_=w1[:].rearrange("(b p) h -> p b h", p=128))
            w2t = cpool.tile([16, H], F32)
            nc.sync.dma_start(out=w2t[:], in_=w2[:])
            a1t = cpool.tile([16, 16], F32)
            nc.sync.dma_start(out=a1t[:], in_=a1rep[:])
            ad1t = cpool.tile([16, 16], F32)
            nc.sync.dma_start(out=ad1t[:], in_=ad1rep[:])
            a2t = cpool.tile([16, 16], F32)
            nc.sync.dma_start(out=a2t[:], in_=a2rep[:])
            ad2t = cpool.tile([16, 16], F32)
            nc.sync.dma_start(out=ad2t[:], in_=ad2rep[:])
            b1t = cpool.tile([16, 1], F32)
            nc.sync.dma_start(out=b1t[:], in_=b1p[:])
            b2t = cpool.tile([16, 1], F32)
            nc.sync.dma_start(out=b2t[:], in_=b2p[:])
            selt = cpool.tile([128, 16], F32)
            nc.sync.dma_start(out=selt[:], in_=selp[:])
            selnt = cpool.tile([128, 16], F32)
            nc.sync.dma_start(out=selnt[:], in_=selnp[:])
            id16t = cpool.tile([16, 16], BF16)
            nc.sync.dma_start(out=id16t[:], in_=id16p[:])
            idt = cpool.tile([16, 16], F32)
            nc.sync.dma_start(out=idt[:], in_=identp[:])
            ones1 = cpool.tile([1, 128], F32)
            nc.vector.memset(ones1[:], 1.0)
            zerob = cpool.tile([128, 1], BF16)
            nc.vector.memset(zerob[:], 0.0)
            zrow = cpool.tile([1, 32], F32)
            nc.vector.memset(zrow[:], 0.0)
            idxt = cpool.tile([128, NT * IW], I16)
            nc.sync.dma_start(out=idxt[:], in_=idxsp[:])

            # ---------------- phase A (tables) -----------------------------
            def phase_a(l, wt, at, adt, rhs_of):
                """Compute per-chunk (h|alpha_src) packed tables, alpha_dst
                row, and packed (wself|pself) self-loop tables; stage to DRAM.
                rhs_of(c, pa, pap) -> ([16, CH] f32 rhs AP, KB') for the
                h matmul: KB' accumulation steps."""
                with (
                    tc.tile_pool(name=f"pa{l}", bufs=2) as pa,
                    tc.tile_pool(name=f"pap{l}", bufs=2, space="PSUM") as pap,
                ):
                    for c in range(NCH):
                        sl = slice(c * CH, (c + 1) * CH)
                        ph = pap.tile([16, CH], F32, tag="ph")
                        rhs_list = rhs_of(c, pa)
                        for b, (lhsT, rhs) in enumerate(rhs_list):
                            nc.tensor.matmul(ph[:], lhsT=lhsT, rhs=rhs,
                                             start=(b == 0),
                                             stop=(b == len(rhs_list) - 1))
                        hch = pa.tile([16, CH], F32, tag="hch")
                        nc.scalar.activation(hch[:], ph[:], AF.Copy)
                        pas = pap.tile([16, CH], F32, tag="pas")
                        nc.tensor.matmul(pas[:], lhsT=at[:], rhs=hch[:],
                                         start=True, stop=True)
                        pad_ = pap.tile([16, CH], F32, tag="pad")
                        nc.tensor.matmul(pad_[:], lhsT=adt[:], rhs=hch[:],
                                         start=True, stop=True)
                        adfull = pa.tile([16, CH], F32, tag="adfull")
                        nc.vector.tensor_copy(adfull[:], pad_[:])
                        nc.sync.dma_start(out=ad_nl[l][0:1, sl], in_=adfull[0:1, :])
                        packed = pa.tile([16, CH], F32, tag="packed")
                        pb = packed[:].bitcast(BF16)
                        nc.scalar.activation(pb[:, 0::2], hch[:], AF.Copy)
                        nc.scalar.activation(pb[:, 1::2], pas[:], AF.Copy)
                        nc.sync.dma_start(out=ag_in[l][:, sl], in_=packed[:])
                        # analytic self-loop terms
                        tself = pa.tile([16, CH], F32, tag="tself")
                        nc.vector.tensor_add(tself[:], pas[:], adfull[:])
                        nc.vector.scalar_tensor_tensor(
                            tself[:], tself[:], NEG_SLOPE, tself[:],
                            op0=ALU.mult, op1=ALU.max)
                        pselfc = pa.tile([16, CH], F32, tag="pselfc")
                        nc.scalar.activation(pselfc[:], tself[:], AF.Exp)
                        wselfc = pa.tile([16, CH], F32, tag="wselfc")
                        nc.vector.tensor_mul(wselfc[:], pselfc[:], hch[:])
                        pwc = pa.tile([16, CH], F32, tag="pwc")
                        pwb = pwc[:].bitcast(BF16)
                        nc.vector.tensor_copy(pwb[:, 0::2], wselfc[:])
                        nc.vector.tensor_copy(pwb[:, 1::2], pselfc[:])
                        nc.sync.dma_start(out=pw_nl[l][:, sl], in_=pwc[:])
                    nc.sync.dma_start(out=ad_nl[l][0:1, NL:], in_=zrow[:])

            def rhs_a1(c, pa):
                xt_t = pa.tile([128, KB, CH], F32, tag="xt")
                nc.sync.dma_start(
                    out=xt_t[:],
                    in_=xth[:, c * KB * CH:(c + 1) * KB * CH]
                    .rearrange("p (b n) -> p b n", b=KB))
                return [(w1t[:, b, :], xt_t[:, b, :]) for b in range(KB)]

            # ---------------- edge phase -----------------------------------
            def run_layer(l, uv_writer):
                nc.gpsimd.collective_compute(
                    "AllGather", ALU.bypass, replica_groups=rg,
                    ins=[ag_in[l][:]], outs=[ag_out[l][:]])
                with tc.tile_pool(name=f"tab{l}", bufs=1) as tabp:
                    table = tabp.tile([128, NLP], F32, tag="table")
                    nc.sync.dma_start(out=table[:, :NL], in_=ag_out[l][:])
                    nc.vector.memset(table[:, NL:], SENTINEL_PACKED)
                    with (
                        tc.tile_pool(name=f"ed{l}", bufs=2) as ed,
                        tc.tile_pool(name=f"eb{l}", bufs=3) as eb,
                        tc.tile_pool(name=f"edp{l}", bufs=2, space="PSUM") as edp,
                        tc.tile_pool(name=f"adp{l}", bufs=3, space="PSUM") as adp,
                    ):
                        def mk_adbs(t):
                            # alpha_dst block row -> PE rank-1 broadcast ->
                            # ACT copy to a dedicated SBUF block table.
                            ads = eb.tile([1, ADW], F32, tag="ads")
                            nc.sync.dma_start(
                                out=ads[:],
                                in_=ad_nl[l][0:1, t * NB: t * NB + ADW])
                            pws = eb.tile([16, NB], F32, tag="pws")
                            nc.sync.dma_start(
                                out=pws[:],
                                in_=pw_nl[l][:, t * NB:(t + 1) * NB])
                            adp_t = adp.tile([128, ADW], F32, tag="adb")
                            nc.tensor.matmul(
                                adp_t[:], lhsT=ones1[:], rhs=ads[:],
                                start=True, stop=True)
                            adbs = eb.tile([128, ADW], F32, tag="adbs")
                            nc.scalar.activation(adbs[:], adp_t[:], AF.Copy)
                            return adbs, pws

                        pws_q = [mk_adbs(0)]
                        if NT > 1:
                            pws_q.append(mk_adbs(1))
                        for t in range(NT):
                            it = idxt[:, t * IW:(t + 1) * IW]
                            adbs, pwsl = pws_q[0]
                            ghs = ed.tile([128, T_e], F32, tag="ghs")
                            nc.gpsimd.ap_gather(
                                ghs[:], table[:], it[:, 0:T16],
                                channels=128, num_elems=NLP, d=1,
                                num_idxs=T_e)
                            adg = ed.tile([128, T_e], F32, tag="adg")
                            nc.gpsimd.ap_gather(
                                adg[:], adbs[:], it[:, T16:2 * T16],
                                channels=128, num_elems=ADW, d=1,
                                num_idxs=T_e)
                            gb = ghs[:].bitcast(BF16)  # [128, 2*T_e]
                            tt = ed.tile([128, T_e], BF16, tag="tt")
                            nc.vector.tensor_add(tt[:], gb[:, 1::2], adg[:])
                            nc.scalar.activation(tt[:], tt[:], AF.Lrelu,
                                                 alpha=NEG_SLOPE)
                            pp = ed.tile([128, T_e], BF16, tag="pp")
                            nc.scalar.activation(pp[:], tt[:], AF.Exp)
                            # prefetch alpha_dst region two tiles ahead
                            if t + 2 < NT:
                                pws_q.append(mk_adbs(t + 2))
                            nc.vector.tensor_mul(tt[:], pp[:], gb[:, 0::2])
                            sc = ed.tile([128, T_e, 2], F32, tag="sc")
                            nc.vector.tensor_tensor_scan(
                                sc[:, :, 0], zerob[:, 0:1].to_broadcast([128, T_e]),
                                tt[:], 0.0, ALU.add, ALU.add)
                            nc.vector.tensor_tensor_scan(
                                sc[:, :, 1], zerob[:, 0:1].to_broadcast([128, T_e]),
                                pp[:], 0.0, ALU.add, ALU.add)
                            bg = ed.tile([128, 1024, 2], F32, tag="bg")
                            nc.gpsimd.ap_gather(
                                bg[:], sc[:], it[:, 2 * T16:2 * T16 + 64],
                                channels=128, num_elems=T_e, d=2, num_idxs=1024)
                            pwb = pwsl[:].bitcast(BF16)  # [16, 2*NB]
                            pu = edp.tile([16, 512], F32, tag="pu")
                            nc.tensor.matmul(pu[:], lhsT=selt[:],
                                             rhs=bg[:, 0:512, 0],
                                             start=True, stop=False)
                            nc.tensor.matmul(pu[:], lhsT=selnt[:],
                                             rhs=bg[:, 512:1024, 0],
                                             start=False, stop=False)
                            nc.tensor.matmul(pu[:, :NB], lhsT=id16t[:],
                                             rhs=pwb[:, 0::2],
                                             start=False, stop=True)
                            ps = edp.tile([16, 512], F32, tag="ps")
                            nc.tensor.matmul(ps[:], lhsT=selt[:],
                                             rhs=bg[:, 0:512, 1],
                                             start=True, stop=False)
                            nc.tensor.matmul(ps[:], lhsT=selnt[:],
                                             rhs=bg[:, 512:1024, 1],
                                             start=False, stop=False)
                            nc.tensor.matmul(ps[:, :NB], lhsT=id16t[:],
                                             rhs=pwb[:, 1::2],
                                             start=False, stop=True)
                            puc = ed.tile([16, NB], F32, tag="puc")
                            nc.scalar.activation(puc[:], pu[:, :NB], AF.Copy)
                            psc = ed.tile([16, NB], F32, tag="psc")
                            nc.scalar.activation(psc[:], ps[:, :NB], AF.Copy)
                            nc.sync.dma_start(out=numd[l][:, t * NB:(t + 1) * NB],
                                              in_=puc[:])
                            nc.sync.dma_start(out=dend[l][:, t * NB:(t + 1) * NB],
                                              in_=psc[:])
                            pws_q.pop(0)

                # batched softmax normalization: uv = num * exp(-ln(den))
                NRM = NL // NRMB
                with tc.tile_pool(name=f"nm{l}", bufs=2) as nmp:
                    for c5 in range(NRMB):
                        sl = slice(c5 * NRM, (c5 + 1) * NRM)
                        nmt = nmp.tile([16, NRM], F32, tag="nmt")
                        nc.sync.dma_start(out=nmt[:], in_=numd[l][:, sl])
                        dnt = nmp.tile([16, NRM], F32, tag="dnt")
                        nc.sync.dma_start(out=dnt[:], in_=dend[l][:, sl])
                        ld = nmp.tile([16, NRM], F32, tag="ld")
                        nc.scalar.activation(ld[:], dnt[:], AF.Ln)
                        rc = nmp.tile([16, NRM], F32, tag="rc")
                        nc.scalar.activation(rc[:], ld[:], AF.Exp, scale=-1.0)
                        uvm = nmp.tile([16, NRM], F32, tag="uvm")
                        nc.vector.tensor_mul(uvm[:], nmt[:], rc[:])
                        uvw = nmp.tile([16, NRM], F32, tag="uvw")
                        uv_writer(uvw, uvm)
                        nc.sync.dma_start(out=uvd[l][:, sl], in_=uvw[:])

            # ======================= layer 1 ===============================
            phase_a(0, w1t, a1t, ad1t, rhs_a1)

            def write1(uvw, uvm):
                nc.vector.tensor_scalar(
                    uvw[:], uvm[:], b1t[:, 0:1], 0.0,
                    op0=ALU.add, op1=ALU.max)

            run_layer(0, write1)

            # phase A (layer 2) reads uv1 chunks back from DRAM
            def rhs_a2(c, pa):
                uc = pa.tile([16, CH], F32, tag="uc")
                nc.sync.dma_start(out=uc[:], in_=uvd[0][:, c * CH:(c + 1) * CH])
                return [(w2t[:], uc[:])]

            phase_a(1, w2t, a2t, ad2t, rhs_a2)

            # ======================= layer 2 ===============================
            def write2(uvw, uvm):
                nc.vector.tensor_scalar_add(uvw[:], uvm[:], b2t[:, 0:1])

            run_layer(1, write2)

            # ---------------- log_softmax + transpose + store -------------
            with (
                tc.tile_pool(name="fin", bufs=2) as fin,
                tc.tile_pool(name="finp", bufs=4, space="PSUM") as finp,
                tc.tile_pool(name="fino", bufs=1) as fino,
            ):
                uv2 = fino.tile([16, NL2], F32, tag="uv2")
                if NL2 > NL:
                    nc.vector.memset(uv2[:, NL:], 0.0)
                nc.sync.dma_start(out=uv2[:, :NL], in_=uvd[1][:])
                if True:
                    nodemaj = fino.tile([128, NBLK, H], F32, tag="nodemaj")
                    for j in range(NBLK):
                        ptp = finp.tile([128, 16], F32, tag="ptp")
                        nc.tensor.transpose(ptp[:], uv2[:, j * 128:(j + 1) * 128],
                                            idt[:])
                        nc.vector.tensor_copy(nodemaj[:, j, :], ptp[:, :H])
                    mx = fin.tile([128, NBLK], F32, tag="mx")
                    nc.vector.tensor_reduce(mx[:], nodemaj[:],
                                            axis=mybir.AxisListType.X,
                                            op=ALU.max)
                    zz = fino.tile([128, NBLK, H], F32, tag="zz")
                    nc.vector.tensor_sub(zz[:], nodemaj[:],
                                         mx[:, :, None].to_broadcast([128, NBLK, H]))
                    es = fino.tile([128, NBLK, H], F32, tag="es")
                    nc.scalar.activation(es[:], zz[:], AF.Exp)
                    sm = fin.tile([128, NBLK], F32, tag="sm")
                    nc.vector.tensor_reduce(sm[:], es[:],
                                            axis=mybir.AxisListType.X,
                                            op=ALU.add)
                    ls = fin.tile([128, NBLK], F32, tag="ls")
                    nc.scalar.activation(ls[:], sm[:], AF.Ln)
                    outf = fino.tile([128, NBLK, H], F32, tag="outf")
                    nc.vector.tensor_sub(outf[:], zz[:],
                                         ls[:, :, None].to_broadcast([128, NBLK, H]))
                    nc.sync.dma_start(out=outp[:].rearrange("p (b h) -> p b h", h=H),
                                      in_=outf[:])

    nc.compile()
    if split:
        split_waits(nc, max_waits=max_waits, ctrl_max_waits=ctrl_max_waits)
    return nc


CTRL_TYPES = ("InstDrain", "InstNoOp", "InstHalt", "InstEventSemaphore")


def split_waits(nc, max_waits=2, ctrl_max_waits=1):
    """walrus in this container caps sync-waits per instruction; move excess
    waits onto preceding same-engine NoOps (each carrying one wait)."""
    for f in nc.m.functions:
        for bb in f.blocks:
            new_insts, changed = [], False
            for ins in bb.instructions:
                si = ins.sync_info
                cap = (ctrl_max_waits if type(ins).__name__ in CTRL_TYPES
                       else max_waits)
                if si is not None and si.on_wait is not None and len(si.on_wait) > cap:
                    waits = list(si.on_wait)
                    excess, keep = waits[:-cap] if cap else waits, waits[-cap:] if cap else []
                    for i, w in enumerate(excess):
                        nop = mybir.InstNoOp(name=f"{ins.name}-ws{i}", ins=[], outs=[])
                        nop.engine = ins.engine
                        nop.sync_info = mybir.SyncInfo(on_wait=[w], on_update=[])
                        new_insts.append(nop)
                    si.on_wait = keep
                    changed = True
                new_insts.append(ins)
            if changed:
                bb.instructions = new_insts
    for f in nc.m.functions:
        for bb in f.blocks:
            for ins in bb.instructions:
                si = ins.sync_info
                cap = (ctrl_max_waits if type(ins).__name__ in CTRL_TYPES
                       else max_waits)
                assert si is None or si.on_wait is None or len(si.on_wait) <= cap, \
                    f"{ins.name}: {len(si.on_wait)} waits > {cap}"


# ------------------------------------------------------------ input packing

def make_in_maps(inputs, cfg, per_core):
    NC, NL, H, F = cfg["NCORES"], cfg["NL"], cfg["H"], cfg["F"]
    CH, NCH = cfg["CH"], cfg["NCH"]
    KB = F // 128
    x = np.asarray(inputs["x"], dtype=np.float32)
    # per-core layout [128, NCH, KB, CH]: xt_h[p, c, b, n] = x[n_glob, 128b+p]
    xt_full = np.ascontiguousarray(x.T)  # [F, N]
    sel = np.zeros((128, 16), dtype=np.float32)
    sel[np.arange(128), np.arange(128) % 16] = 1.0
    shared = {
        "w1": np.ascontiguousarray(np.asarray(inputs["W1"], np.float32)),
        "w2": np.ascontiguousarray(np.asarray(inputs["W2"], np.float32)),
        "a1rep": np.ascontiguousarray(np.repeat(np.asarray(inputs["a_src1"], np.float32)[:, None], 16, 1)),
        "ad1rep": np.ascontiguousarray(np.repeat(np.asarray(inputs["a_dst1"], np.float32)[:, None], 16, 1)),
        "a2rep": np.ascontiguousarray(np.repeat(np.asarray(inputs["a_src2"], np.float32)[:, None], 16, 1)),
        "ad2rep": np.ascontiguousarray(np.repeat(np.asarray(inputs["a_dst2"], np.float32)[:, None], 16, 1)),
        "b1p": np.ascontiguousarray(np.asarray(inputs["b1"], np.float32)[:, None]),
        "b2p": np.ascontiguousarray(np.asarray(inputs["b2"], np.float32)[:, None]),
        "selp": sel,
        "selnp": -sel,
        "id16p": np.eye(16, dtype=ml_dtypes.bfloat16),
        "identp": np.eye(16, dtype=np.float32),
    }
    in_maps = []
    for c in range(NC):
        m = dict(shared)
        xc = xt_full[:, c * NL:(c + 1) * NL]          # [F, NL]
        xc = xc.reshape(KB, 128, NCH, CH)             # [b, p, c, n]
        xc = xc.transpose(1, 2, 0, 3)                 # [p, c, b, n]
        m["xth"] = np.ascontiguousarray(xc.reshape(128, NCH * KB * CH))
        m["idxs"] = per_core[c]["idxs"]
        in_maps.append(m)
    return in_maps


def unshard_output(results, cfg):
    NC, NL, H = cfg["NCORES"], cfg["NL"], cfg["H"]
    NBLK = math.ceil(NL / 128)
    parts = []
    for c in range(NC):
        a = np.asarray(results[c]["out"]).reshape(128, NBLK, H)
        a = a.transpose(1, 0, 2).reshape(NBLK * 128, H)[:NL]
        parts.append(a)
    return np.concatenate(parts, axis=0)


# ------------------------------------------------------------------- driver

_CACHE = {}


def run_on_hw(inputs, cfg, trace=False, tmpdir=None):
    import os
    import shutil
    from concourse.bass_utils import run_bass_kernel_spmd
    if tmpdir is not None and os.path.isdir(tmpdir):
        shutil.rmtree(tmpdir, ignore_errors=True)
    if tmpdir is not None:
        os.makedirs(tmpdir, exist_ok=True)
    T_e, per_core = host_prep(inputs["edge_index"], cfg)
    key = (cfg["N"], T_e)
    if key not in _CACHE:
        _CACHE[key] = build_nc(cfg, T_e)
    nc = _CACHE[key]
    in_maps = make_in_maps(inputs, cfg, per_core)
    res = run_bass_kernel_spmd(nc, in_maps, list(range(cfg["NCORES"])),
                               trace=trace, tmpdir=tmpdir)
    out = unshard_output(res.results, cfg)
    return out, res


def kernel(**inputs):
    out, _ = run_on_hw(inputs, FULL_CFG)
    return out.astype(np.float32)


# revision 22
# speedup vs baseline: 1.3544x; 1.1394x over previous
"""GAT (2-layer, heads=1) on 8 Trainium2 NeuronCores.

Strategy (1D node partition): nodes split into 8 chunks of NL; core c owns
dst-chunk c. Per layer, node tables (h|alpha_src packed 2xbf16 per fp32 word)
are computed feature-major via PE matmuls, AllGathered, and held in SBUF:
partition group g (16 partitions) holds chunk g's table. Edges are bucketed
on the host by (dst-core, src-chunk, dst-block) and sorted by dst; each edge
tile covers one NB-wide dst block for all 8 src-chunk groups. Per-edge
gathers run on GPSIMD (ap_gather), the logit add + exp-weighted h on DVE/ACT,
and segment sums via UNMASKED fp32 prefix scans + boundary-difference:
cum[last(d)] - cum[first(d)-1], folded into the PE group-combine as
(+sel)x(end) + (-sel)x(begin) PSUM accumulation; self-loop terms are
PE-accumulated from precomputed per-node tables. Per-node softmax
normalization (reciprocal) is batched per layer as ACT ln/exp passes.
Host preprocessing only reorders/buckets edge indices and relayouts x.
"""

import math
import numpy as np

import ml_dtypes

from concourse import bass, bacc, mybir
import concourse.tile as tile

F32 = mybir.dt.float32
BF16 = mybir.dt.bfloat16
I16 = mybir.dt.int16

NEG_SLOPE = 0.2
# fp32 -29952.0 == 0xC6EA0000: low bf16 lane (h) = 0, high bf16 lane
# (alpha_src) = -29952 -> exp(lrelu(...)) == 0, so sentinel edges vanish.
SENTINEL_PACKED = -29952.0

FULL_CFG = dict(
    NCORES=8, N=100000, F=512, H=16,
    NL=12500, NB=500, NT=25, CH=500, NCH=25,
)

SMALL_CFG = dict(
    NCORES=8, N=12800, F=512, H=16,
    NL=1600, NB=400, NT=4, CH=400, NCH=4,
)


# ---------------------------------------------------------------- host prep

def _round_up(x, m):
    return (x + m - 1) // m * m


def host_prep(edge_index, cfg):
    """Dst-major K-padded edge layout.

    Nodes are permuted block-locally so each NB-block's dsts are ordered by
    degree class K in {8,16,32} (K >= max over the 8 src-chunk groups of the
    dst's in-degree from that group). Row r of a tile owns K(r) consecutive
    slots; each (group, dst) fills its edges into its row's slots (sentinel
    NL pads). Segment sums then become strided tensor_reduce - no scans, no
    boundary gathers, and one src-table gather per tile.

    Returns (layout, per_core, order) where layout = (S, A, B, C) row-class
    counts, per_core[c]['idxs'] = [128, NT*S16] i16, and order[i] = original
    node id at permuted position i.
    """
    NC, N, NL = cfg["NCORES"], cfg["N"], cfg["NL"]
    NB, NT = cfg["NB"], cfg["NT"]
    G = 8

    src = np.asarray(edge_index[0], dtype=np.int64)
    dst = np.asarray(edge_index[1], dtype=np.int64)
    E = src.size

    grp = src // NL
    deg_gd = np.bincount(grp * N + dst, minlength=G * N).reshape(G, N)
    D = deg_gd.max(axis=0)
    assert D.max() <= 32, D.max()
    cls = (D > 8).astype(np.int64) + (D > 16)

    blk_id = np.arange(N) // NB
    order = np.lexsort((np.arange(N), cls, blk_id))
    newpos = np.empty(N, dtype=np.int64)
    newpos[order] = np.arange(N)

    nblk = N // NB
    n2 = np.bincount(blk_id[cls == 2], minlength=nblk)
    n12 = np.bincount(blk_id[cls >= 1], minlength=nblk)
    C = int(n2.max())
    B = max(int(n12.max()) - C, 0)
    A = NB - B - C
    if A % 2 == 1:
        A -= 1
        B += 1
    assert A >= 0
    S = 8 * A + 16 * B + 32 * C
    assert S % 16 == 0

    Krow = np.full(NB, 8, dtype=np.int64)
    Krow[A:A + B] = 16
    Krow[A + B:] = 32
    offrow = np.zeros(NB, dtype=np.int64)
    offrow[1:] = np.cumsum(Krow)[:-1]

    newd = newpos[dst]
    news = newpos[src]
    core = newd // NL
    t_ = (newd % NL) // NB
    r_ = newd % NB
    # per-(grp, dst) edge rank
    key = grp * N + newd
    oe = np.lexsort((np.arange(E), key))
    ke = key[oe]
    first = np.ones(E, dtype=bool)
    first[1:] = ke[1:] != ke[:-1]
    startpos = np.where(first, np.arange(E), 0)
    j_sorted = np.arange(E) - np.maximum.accumulate(startpos)
    j = np.empty(E, dtype=np.int64)
    j[oe] = j_sorted
    assert (j < Krow[r_]).all()
    slot = offrow[r_] + j

    srcl = np.full((NC, G, NT, S), NL, dtype=np.int16)
    srcl[core, grp, t_, slot] = (news % NL).astype(np.int16)

    S16 = S // 16
    idxs = (srcl.reshape(NC, G, NT, S16, 16)
            .transpose(0, 1, 4, 2, 3)
            .reshape(NC, 128, NT * S16))
    idxs = np.ascontiguousarray(idxs)
    per_core = [{"idxs": idxs[c]} for c in range(NC)]
    return (S, A, B, C), per_core, order


# ------------------------------------------------------------- device build

def build_nc(cfg, layout, max_waits=2, ctrl_max_waits=1, split=True):
    NC, N, F, H = cfg["NCORES"], cfg["N"], cfg["F"], cfg["H"]
    NL, NB, NT = cfg["NL"], cfg["NB"], cfg["NT"]
    CH, NCH = cfg["CH"], cfg["NCH"]
    S, RA, RB, RC = layout
    KB = F // 128
    NLP = NL + 16               # table width incl. sentinel
    ADW = NB + 4                # alpha_dst row width
    S16 = S // 16
    # (slot offset, row count, K, row offset) per degree-class region
    REGIONS = [(0, RA, 8, 0), (8 * RA, RB, 16, RA),
               (8 * RA + 16 * RB, RC, 32, RA + RB)]
    REGIONS = [rg_ for rg_ in REGIONS if rg_[1] > 0]
    NBLK = math.ceil(NL / 128)
    NL2 = NBLK * 128
    NRMB = 5                    # chunks in the normalization pass
    assert NL % NRMB == 0
    rg = [list(range(NC))]

    nc = bacc.Bacc("TRN2", target_bir_lowering=False)

    xth = nc.declare_dram_parameter("xth", [128, NCH * KB * CH], F32, isOutput=False)
    w1 = nc.declare_dram_parameter("w1", [F, H], F32, isOutput=False)
    w2 = nc.declare_dram_parameter("w2", [H, H], F32, isOutput=False)
    a1rep = nc.declare_dram_parameter("a1rep", [H, 16], F32, isOutput=False)
    ad1rep = nc.declare_dram_parameter("ad1rep", [H, 16], F32, isOutput=False)
    a2rep = nc.declare_dram_parameter("a2rep", [H, 16], F32, isOutput=False)
    ad2rep = nc.declare_dram_parameter("ad2rep", [H, 16], F32, isOutput=False)
    b1p = nc.declare_dram_parameter("b1p", [H, 1], F32, isOutput=False)
    b2p = nc.declare_dram_parameter("b2p", [H, 1], F32, isOutput=False)
    selp = nc.declare_dram_parameter("selp", [128, 16], F32, isOutput=False)
    selnp = nc.declare_dram_parameter("selnp", [128, 16], F32, isOutput=False)
    id16p = nc.declare_dram_parameter("id16p", [16, 16], BF16, isOutput=False)
    identp = nc.declare_dram_parameter("identp", [16, 16], F32, isOutput=False)
    idxsp = nc.declare_dram_parameter("idxs", [128, NT * S16], I16, isOutput=False)
    outp = nc.declare_dram_parameter("out", [128, NBLK * H], F32, isOutput=True)

    ag_in = [nc.dram_tensor(f"ag_in{l}", [16, NL], F32) for l in (1, 2)]
    ag_out = [nc.dram_tensor(f"ag_out{l}", [128, NL], F32, addr_space="Shared")
              for l in (1, 2)]
    ad_nl = [nc.dram_tensor(f"ad_nl{l}", [1, NL + 32], F32) for l in (1, 2)]
    pw_nl = [nc.dram_tensor(f"pw_nl{l}", [16, NL], F32) for l in (1, 2)]
    numd = [nc.dram_tensor(f"numd{l}", [16, NL], F32) for l in (1, 2)]
    dend = [nc.dram_tensor(f"dend{l}", [16, NL], F32) for l in (1, 2)]
    uvd = [nc.dram_tensor(f"uvd{l}", [16, NL], F32) for l in (1, 2)]

    AF = mybir.ActivationFunctionType
    ALU = mybir.AluOpType

    with tile.TileContext(nc, num_cores=NC) as tc:
        with tc.tile_pool(name="const", bufs=1) as cpool:
            w1t = cpool.tile([128, KB, H], F32)
            nc.sync.dma_start(out=w1t[:], in_=w1[:].rearrange("(b p) h -> p b h", p=128))
            w2t = cpool.tile([16, H], F32)
            nc.sync.dma_start(out=w2t[:], in_=w2[:])
            a1t = cpool.tile([16, 16], F32)
            nc.sync.dma_start(out=a1t[:], in_=a1rep[:])
            ad1t = cpool.tile([16, 16], F32)
            nc.sync.dma_start(out=ad1t[:], in_=ad1rep[:])
            a2t = cpool.tile([16, 16], F32)
            nc.sync.dma_start(out=a2t[:], in_=a2rep[:])
            ad2t = cpool.tile([16, 16], F32)
            nc.sync.dma_start(out=ad2t[:], in_=ad2rep[:])
            b1t = cpool.tile([16, 1], F32)
            nc.sync.dma_start(out=b1t[:], in_=b1p[:])
            b2t = cpool.tile([16, 1], F32)
            nc.sync.dma_start(out=b2t[:], in_=b2p[:])
            selt = cpool.tile([128, 16], F32)
            nc.sync.dma_start(out=selt[:], in_=selp[:])
            selnt = cpool.tile([128, 16], F32)
            nc.sync.dma_start(out=selnt[:], in_=selnp[:])
            id16t = cpool.tile([16, 16], BF16)
            nc.sync.dma_start(out=id16t[:], in_=id16p[:])
            idt = cpool.tile([16, 16], F32)
            nc.sync.dma_start(out=idt[:], in_=identp[:])
            ones1 = cpool.tile([1, 128], F32)
            nc.vector.memset(ones1[:], 1.0)
            zrow = cpool.tile([1, 32], F32)
            nc.vector.memset(zrow[:], 0.0)

            # ---------------- phase A (tables) -----------------------------
            def phase_a(l, wt, at, adt, rhs_of):
                """Compute per-chunk (h|alpha_src) packed tables, alpha_dst
                row, and packed (wself|pself) self-loop tables; stage to DRAM.
                rhs_of(c, pa, pap) -> ([16, CH] f32 rhs AP, KB') for the
                h matmul: KB' accumulation steps."""
                with (
                    tc.tile_pool(name=f"pa{l}", bufs=2) as pa,
                    tc.tile_pool(name=f"pap{l}", bufs=2, space="PSUM") as pap,
                ):
                    for c in range(NCH):
                        sl = slice(c * CH, (c + 1) * CH)
                        ph = pap.tile([16, CH], F32, tag="ph")
                        rhs_list = rhs_of(c, pa)
                        for b, (lhsT, rhs) in enumerate(rhs_list):
                            nc.tensor.matmul(ph[:], lhsT=lhsT, rhs=rhs,
                                             start=(b == 0),
                                             stop=(b == len(rhs_list) - 1))
                        hch = pa.tile([16, CH], F32, tag="hch")
                        nc.scalar.activation(hch[:], ph[:], AF.Copy)
                        pas = pap.tile([16, CH], F32, tag="pas")
                        nc.tensor.matmul(pas[:], lhsT=at[:], rhs=hch[:],
                                         start=True, stop=True)
                        pad_ = pap.tile([16, CH], F32, tag="pad")
                        nc.tensor.matmul(pad_[:], lhsT=adt[:], rhs=hch[:],
                                         start=True, stop=True)
                        adfull = pa.tile([16, CH], F32, tag="adfull")
                        nc.vector.tensor_copy(adfull[:], pad_[:])
                        nc.sync.dma_start(out=ad_nl[l][0:1, sl], in_=adfull[0:1, :])
                        packed = pa.tile([16, CH], F32, tag="packed")
                        pb = packed[:].bitcast(BF16)
                        nc.scalar.activation(pb[:, 0::2], hch[:], AF.Copy)
                        nc.scalar.activation(pb[:, 1::2], pas[:], AF.Copy)
                        nc.sync.dma_start(out=ag_in[l][:, sl], in_=packed[:])
                        # analytic self-loop terms
                        tself = pa.tile([16, CH], F32, tag="tself")
                        nc.vector.tensor_add(tself[:], pas[:], adfull[:])
                        nc.vector.scalar_tensor_tensor(
                            tself[:], tself[:], NEG_SLOPE, tself[:],
                            op0=ALU.mult, op1=ALU.max)
                        pselfc = pa.tile([16, CH], F32, tag="pselfc")
                        nc.scalar.activation(pselfc[:], tself[:], AF.Exp)
                        wselfc = pa.tile([16, CH], F32, tag="wselfc")
                        nc.vector.tensor_mul(wselfc[:], pselfc[:], hch[:])
                        pwc = pa.tile([16, CH], F32, tag="pwc")
                        pwb = pwc[:].bitcast(BF16)
                        nc.vector.tensor_copy(pwb[:, 0::2], wselfc[:])
                        nc.vector.tensor_copy(pwb[:, 1::2], pselfc[:])
                        nc.sync.dma_start(out=pw_nl[l][:, sl], in_=pwc[:])
                    nc.sync.dma_start(out=ad_nl[l][0:1, NL:], in_=zrow[:])

            def rhs_a1(c, pa):
                xt_t = pa.tile([128, KB, CH], F32, tag="xt")
                nc.sync.dma_start(
                    out=xt_t[:],
                    in_=xth[:, c * KB * CH:(c + 1) * KB * CH]
                    .rearrange("p (b n) -> p b n", b=KB))
                return [(w1t[:, b, :], xt_t[:, b, :]) for b in range(KB)]

            # ---------------- edge phase -----------------------------------
            def run_layer(l, uv_writer):
                nc.gpsimd.collective_compute(
                    "AllGather", ALU.bypass, replica_groups=rg,
                    ins=[ag_in[l][:]], outs=[ag_out[l][:]])
                batches = [list(range(b, min(b + 2, NT)))
                           for b in range(0, NT, 2)]
                with tc.tile_pool(name=f"tab{l}", bufs=1) as tabp:
                    table = tabp.tile([128, NLP], F32, tag="table")
                    nc.sync.dma_start(out=table[:, :NL], in_=ag_out[l][:])
                    nc.vector.memset(table[:, NL:], SENTINEL_PACKED)
                    with (
                        tc.tile_pool(name=f"ed{l}", bufs=2) as ed,
                        tc.tile_pool(name=f"ei{l}", bufs=3) as ei,
                        tc.tile_pool(name=f"eb{l}", bufs=2) as eb,
                        tc.tile_pool(name=f"er{l}", bufs=2) as er,
                        tc.tile_pool(name=f"edp{l}", bufs=2, space="PSUM") as edp,
                        tc.tile_pool(name=f"adp{l}", bufs=2, space="PSUM") as adp,
                    ):
                        def mk_gather(bi):
                            tiles = batches[bi]
                            w = len(tiles) * S16
                            ib = ei.tile([128, 2 * S16], I16, tag="ib")
                            nc.sync.dma_start(
                                out=ib[:, :w],
                                in_=idxsp[:, tiles[0] * S16:
                                          tiles[0] * S16 + w])
                            ghs = ed.tile([128, 2 * S], F32, tag="ghs")
                            nc.gpsimd.ap_gather(
                                ghs[:, :len(tiles) * S], table[:], ib[:, :w],
                                channels=128, num_elems=NLP, d=1,
                                num_idxs=len(tiles) * S)
                            return ghs

                        def mk_loads(t):
                            ads = eb.tile([1, ADW], F32, tag="ads")
                            nc.sync.dma_start(
                                out=ads[:],
                                in_=ad_nl[l][0:1, t * NB: t * NB + ADW])
                            pws = eb.tile([16, NB], F32, tag="pws")
                            nc.sync.dma_start(
                                out=pws[:],
                                in_=pw_nl[l][:, t * NB:(t + 1) * NB])
                            return ads, pws

                        gq = [mk_gather(0)]
                        if len(batches) > 1:
                            gq.append(mk_gather(1))
                        lq = [mk_loads(0)]
                        for bi, tiles in enumerate(batches):
                            ghs = gq.pop(0)
                            if bi + 2 < len(batches):
                                gq.append(mk_gather(bi + 2))
                            for k, t in enumerate(tiles):
                                ads, pwsl = lq.pop(0)
                                if t + 1 < NT:
                                    lq.append(mk_loads(t + 1))
                                gb = ghs[:, k * S:(k + 1) * S].bitcast(BF16)
                                adps = adp.tile([128, NB], F32, tag="adps")
                                nc.tensor.matmul(adps[:], lhsT=ones1[:],
                                                 rhs=ads[:, :NB],
                                                 start=True, stop=True)
                                tt = ed.tile([128, S], BF16, tag="tt")
                                for so, nr, K, ro in REGIONS:
                                    nc.vector.tensor_add(
                                        tt[:, so:so + nr * K]
                                        .rearrange("p (n k) -> p n k", k=K),
                                        gb[:, 1::2][:, so:so + nr * K]
                                        .rearrange("p (n k) -> p n k", k=K),
                                        adps[:, ro:ro + nr, None]
                                        .to_broadcast([128, nr, K]))
                                nc.scalar.activation(tt[:], tt[:], AF.Lrelu,
                                                     alpha=NEG_SLOPE)
                                nc.scalar.activation(tt[:], tt[:], AF.Exp)
                                ww = ed.tile([128, S], BF16, tag="ww")
                                nc.vector.tensor_mul(ww[:], tt[:], gb[:, 0::2])
                                redw = er.tile([128, NB], F32, tag="redw")
                                redp = er.tile([128, NB], F32, tag="redp")
                                for so, nr, K, ro in REGIONS:
                                    nc.vector.tensor_reduce(
                                        redw[:, ro:ro + nr],
                                        ww[:, so:so + nr * K]
                                        .rearrange("p (n k) -> p n k", k=K),
                                        axis=mybir.AxisListType.X, op=ALU.add)
                                    nc.vector.tensor_reduce(
                                        redp[:, ro:ro + nr],
                                        tt[:, so:so + nr * K]
                                        .rearrange("p (n k) -> p n k", k=K),
                                        axis=mybir.AxisListType.X, op=ALU.add)
                                pwb = pwsl[:].bitcast(BF16)  # [16, 2*NB]
                                pu = edp.tile([16, NB], F32, tag="pu")
                                nc.tensor.matmul(pu[:], lhsT=selt[:],
                                                 rhs=redw[:],
                                                 start=True, stop=False)
                                nc.tensor.matmul(pu[:], lhsT=id16t[:],
                                                 rhs=pwb[:, 0::2],
                                                 start=False, stop=True)
                                ps = edp.tile([16, NB], F32, tag="ps")
                                nc.tensor.matmul(ps[:], lhsT=selt[:],
                                                 rhs=redp[:],
                                                 start=True, stop=False)
                                nc.tensor.matmul(ps[:], lhsT=id16t[:],
                                                 rhs=pwb[:, 1::2],
                                                 start=False, stop=True)
                                puc = er.tile([16, NB], F32, tag="puc")
                                nc.scalar.activation(puc[:], pu[:], AF.Copy)
                                psc = er.tile([16, NB], F32, tag="psc")
                                nc.scalar.activation(psc[:], ps[:], AF.Copy)
                                nc.sync.dma_start(
                                    out=numd[l][:, t * NB:(t + 1) * NB],
                                    in_=puc[:])
                                nc.sync.dma_start(
                                    out=dend[l][:, t * NB:(t + 1) * NB],
                                    in_=psc[:])

                # batched softmax normalization: uv = num * exp(-ln(den))
                NRM = NL // NRMB
                with tc.tile_pool(name=f"nm{l}", bufs=2) as nmp:
                    for c5 in range(NRMB):
                        sl = slice(c5 * NRM, (c5 + 1) * NRM)
                        nmt = nmp.tile([16, NRM], F32, tag="nmt")
                        nc.sync.dma_start(out=nmt[:], in_=numd[l][:, sl])
                        dnt = nmp.tile([16, NRM], F32, tag="dnt")
                        nc.sync.dma_start(out=dnt[:], in_=dend[l][:, sl])
                        ld = nmp.tile([16, NRM], F32, tag="ld")
                        nc.scalar.activation(ld[:], dnt[:], AF.Ln)
                        rc = nmp.tile([16, NRM], F32, tag="rc")
                        nc.scalar.activation(rc[:], ld[:], AF.Exp, scale=-1.0)
                        uvm = nmp.tile([16, NRM], F32, tag="uvm")
                        nc.vector.tensor_mul(uvm[:], nmt[:], rc[:])
                        uvw = nmp.tile([16, NRM], F32, tag="uvw")
                        uv_writer(uvw, uvm)
                        nc.sync.dma_start(out=uvd[l][:, sl], in_=uvw[:])

            # ======================= layer 1 ===============================
            phase_a(0, w1t, a1t, ad1t, rhs_a1)

            def write1(uvw, uvm):
                nc.vector.tensor_scalar(
                    uvw[:], uvm[:], b1t[:, 0:1], 0.0,
                    op0=ALU.add, op1=ALU.max)

            run_layer(0, write1)

            # phase A (layer 2) reads uv1 chunks back from DRAM
            def rhs_a2(c, pa):
                uc = pa.tile([16, CH], F32, tag="uc")
                nc.sync.dma_start(out=uc[:], in_=uvd[0][:, c * CH:(c + 1) * CH])
                return [(w2t[:], uc[:])]

            phase_a(1, w2t, a2t, ad2t, rhs_a2)

            # ======================= layer 2 ===============================
            def write2(uvw, uvm):
                nc.vector.tensor_scalar_add(uvw[:], uvm[:], b2t[:, 0:1])

            run_layer(1, write2)

            # ---------------- log_softmax + transpose + store -------------
            with (
                tc.tile_pool(name="fin", bufs=2) as fin,
                tc.tile_pool(name="finp", bufs=4, space="PSUM") as finp,
                tc.tile_pool(name="fino", bufs=1) as fino,
            ):
                uv2 = fino.tile([16, NL2], F32, tag="uv2")
                if NL2 > NL:
                    nc.vector.memset(uv2[:, NL:], 0.0)
                nc.sync.dma_start(out=uv2[:, :NL], in_=uvd[1][:])
                if True:
                    nodemaj = fino.tile([128, NBLK, H], F32, tag="nodemaj")
                    for j in range(NBLK):
                        ptp = finp.tile([128, 16], F32, tag="ptp")
                        nc.tensor.transpose(ptp[:], uv2[:, j * 128:(j + 1) * 128],
                                            idt[:])
                        nc.vector.tensor_copy(nodemaj[:, j, :], ptp[:, :H])
                    mx = fin.tile([128, NBLK], F32, tag="mx")
                    nc.vector.tensor_reduce(mx[:], nodemaj[:],
                                            axis=mybir.AxisListType.X,
                                            op=ALU.max)
                    zz = fino.tile([128, NBLK, H], F32, tag="zz")
                    nc.vector.tensor_sub(zz[:], nodemaj[:],
                                         mx[:, :, None].to_broadcast([128, NBLK, H]))
                    es = fino.tile([128, NBLK, H], F32, tag="es")
                    nc.scalar.activation(es[:], zz[:], AF.Exp)
                    sm = fin.tile([128, NBLK], F32, tag="sm")
                    nc.vector.tensor_reduce(sm[:], es[:],
                                            axis=mybir.AxisListType.X,
                                            op=ALU.add)
                    ls = fin.tile([128, NBLK], F32, tag="ls")
                    nc.scalar.activation(ls[:], sm[:], AF.Ln)
                    outf = fino.tile([128, NBLK, H], F32, tag="outf")
                    nc.vector.tensor_sub(outf[:], zz[:],
                                         ls[:, :, None].to_broadcast([128, NBLK, H]))
                    nc.sync.dma_start(out=outp[:].rearrange("p (b h) -> p b h", h=H),
                                      in_=outf[:])

    nc.compile()
    if split:
        split_waits(nc, max_waits=max_waits, ctrl_max_waits=ctrl_max_waits)
    return nc


CTRL_TYPES = ("InstDrain", "InstNoOp", "InstHalt", "InstEventSemaphore")


def split_waits(nc, max_waits=2, ctrl_max_waits=1):
    """walrus in this container caps sync-waits per instruction; move excess
    waits onto preceding same-engine NoOps (each carrying one wait)."""
    for f in nc.m.functions:
        for bb in f.blocks:
            new_insts, changed = [], False
            for ins in bb.instructions:
                si = ins.sync_info
                cap = (ctrl_max_waits if type(ins).__name__ in CTRL_TYPES
                       else max_waits)
                if si is not None and si.on_wait is not None and len(si.on_wait) > cap:
                    waits = list(si.on_wait)
                    excess, keep = waits[:-cap] if cap else waits, waits[-cap:] if cap else []
                    for i, w in enumerate(excess):
                        nop = mybir.InstNoOp(name=f"{ins.name}-ws{i}", ins=[], outs=[])
                        nop.engine = ins.engine
                        nop.sync_info = mybir.SyncInfo(on_wait=[w], on_update=[])
                        new_insts.append(nop)
                    si.on_wait = keep
                    changed = True
                new_insts.append(ins)
            if changed:
                bb.instructions = new_insts
    for f in nc.m.functions:
        for bb in f.blocks:
            for ins in bb.instructions:
                si = ins.sync_info
                cap = (ctrl_max_waits if type(ins).__name__ in CTRL_TYPES
                       else max_waits)
                assert si is None or si.on_wait is None or len(si.on_wait) <= cap, \
                    f"{ins.name}: {len(si.on_wait)} waits > {cap}"


# ------------------------------------------------------------ input packing

def make_in_maps(inputs, cfg, per_core, order):
    NC, NL, H, F = cfg["NCORES"], cfg["NL"], cfg["H"], cfg["F"]
    CH, NCH = cfg["CH"], cfg["NCH"]
    KB = F // 128
    x = np.asarray(inputs["x"], dtype=np.float32)[order]  # permuted node order
    # per-core layout [128, NCH, KB, CH]: xt_h[p, c, b, n] = x[n_glob, 128b+p]
    xt_full = np.ascontiguousarray(x.T)  # [F, N]
    sel = np.zeros((128, 16), dtype=np.float32)
    sel[np.arange(128), np.arange(128) % 16] = 1.0
    shared = {
        "w1": np.ascontiguousarray(np.asarray(inputs["W1"], np.float32)),
        "w2": np.ascontiguousarray(np.asarray(inputs["W2"], np.float32)),
        "a1rep": np.ascontiguousarray(np.repeat(np.asarray(inputs["a_src1"], np.float32)[:, None], 16, 1)),
        "ad1rep": np.ascontiguousarray(np.repeat(np.asarray(inputs["a_dst1"], np.float32)[:, None], 16, 1)),
        "a2rep": np.ascontiguousarray(np.repeat(np.asarray(inputs["a_src2"], np.float32)[:, None], 16, 1)),
        "ad2rep": np.ascontiguousarray(np.repeat(np.asarray(inputs["a_dst2"], np.float32)[:, None], 16, 1)),
        "b1p": np.ascontiguousarray(np.asarray(inputs["b1"], np.float32)[:, None]),
        "b2p": np.ascontiguousarray(np.asarray(inputs["b2"], np.float32)[:, None]),
        "selp": sel,
        "selnp": -sel,
        "id16p": np.eye(16, dtype=ml_dtypes.bfloat16),
        "identp": np.eye(16, dtype=np.float32),
    }
    in_maps = []
    for c in range(NC):
        m = dict(shared)
        xc = xt_full[:, c * NL:(c + 1) * NL]          # [F, NL]
        xc = xc.reshape(KB, 128, NCH, CH)             # [b, p, c, n]
        xc = xc.transpose(1, 2, 0, 3)                 # [p, c, b, n]
        m["xth"] = np.ascontiguousarray(xc.reshape(128, NCH * KB * CH))
        m["idxs"] = per_core[c]["idxs"]
        in_maps.append(m)
    return in_maps


def unshard_output(results, cfg, order):
    NC, NL, H = cfg["NCORES"], cfg["NL"], cfg["H"]
    NBLK = math.ceil(NL / 128)
    parts = []
    for c in range(NC):
        a = np.asarray(results[c]["out"]).reshape(128, NBLK, H)
        a = a.transpose(1, 0, 2).reshape(NBLK * 128, H)[:NL]
        parts.append(a)
    out_perm = np.concatenate(parts, axis=0)  # rows = permuted positions
    out = np.empty_like(out_perm)
    out[order] = out_perm
    return out


# ------------------------------------------------------------------- driver

_CACHE = {}


def run_on_hw(inputs, cfg, trace=False, tmpdir=None):
    import os
    import shutil
    from concourse.bass_utils import run_bass_kernel_spmd
    if tmpdir is not None and os.path.isdir(tmpdir):
        shutil.rmtree(tmpdir, ignore_errors=True)
    if tmpdir is not None:
        os.makedirs(tmpdir, exist_ok=True)
    layout, per_core, order = host_prep(inputs["edge_index"], cfg)
    key = (cfg["N"],) + tuple(layout)
    if key not in _CACHE:
        _CACHE[key] = build_nc(cfg, layout)
    nc = _CACHE[key]
    in_maps = make_in_maps(inputs, cfg, per_core, order)
    res = run_bass_kernel_spmd(nc, in_maps, list(range(cfg["NCORES"])),
                               trace=trace, tmpdir=tmpdir)
    out = unshard_output(res.results, cfg, order)
    return out, res


def kernel(**inputs):
    out, _ = run_on_hw(inputs, FULL_CFG)
    return out.astype(np.float32)
